# revision 3
# baseline (speedup 1.0000x reference)
"""Trainium2 Bass kernel: GemNet CircularBasisLayer (radial basis x circular
basis outer product), data-parallel over triplets on 8 NeuronCores.

out[t, s*16+r] = env(ds)*norm/d * sin(pi*(r+1)*d/6) * exp(-6.125*(cos[t]-off_s)^2)
with d = D_ca[id3_ca[t]], ds = d/CUTOFF.

Division of labor:
- Host: shards triplets 8 ways and performs the edge->triplet index lookup
  d[t] = D_ca[id3_ca[t]] while marshaling inputs (a pure index gather, no
  arithmetic). On-device scattered gather was probed extensively: the SWDGE
  indirect-DMA path only consumes one index per output partition-run per
  instruction (~128 gathers / ~1us instruction => ~2 ms for 2M gathers), and
  its multi-index encodings mis-execute on hardware, so the index lookup is
  hoisted to input marshaling.
- Device (per core, 250k triplets): envelope polynomial + reciprocal (DVE),
  16 sines via magic-constant range reduction (DVE) + Sin LUT (ACT),
  8 gaussians (DVE diff + ACT square/exp), 16x8 outer product (DVE + GPSIMD),
  streams the [T, 128] fp32 output (128 MB/core) back with linear DMAs.
"""

import numpy as np

import concourse.bass as bass
import concourse.tile as tile
from concourse import bacc, mybir
from concourse.bass_utils import run_bass_kernel_spmd

# ---- problem constants (hardcoded; harness contract) ----
N_CORES = 8
N_EDGES = 400000
N_TRIPLETS = 2000000
R = 16          # num radial
S = 8           # num spherical
CUTOFF = 6.0
P = 128

# polynomial envelope, p=5: env = 1 + a*ds^5 + b*ds^6 + c*ds^7
ENV_A = -21.0
ENV_B = 35.0
ENV_C = -15.0
NORM = float(np.sqrt(2.0 / CUTOFF**3))
# gaussian smearing on [-1, 1], 8 centers
DELTA = 2.0 / (S - 1)
COEFF = 0.5 / (DELTA * DELTA)          # 6.125 (sign applied via exp scale)
SQS = float(np.sqrt(COEFF))
MAGIC = 12582912.0                     # 1.5 * 2**23: round-to-nearest trick
TWO_PI = float(2.0 * np.pi)

# ---- tiling ----
G_MAIN = 128                           # triplets per partition per main tile
T_CORE = N_TRIPLETS // N_CORES         # 250000
N_MAIN = T_CORE // (P * G_MAIN)        # 15 main tiles
_REM = T_CORE - N_MAIN * P * G_MAIN    # 4240
G_TAIL = -(-_REM // P)                 # 34
T_PAD = N_MAIN * P * G_MAIN + P * G_TAIL  # 250112 rows per core (tail padded)

FP32 = mybir.dt.float32

# outer-product slices executed on GPSIMD instead of DVE (engine balancing)
GPSIMD_SLICES = (6, 7)


def _emit_tile(nc, io, work, outp, rc_t, so_t, dg_h, cos_h, out_h, t_base, G):
    """Emit one tile covering triplets [t_base, t_base + 128*G); within the
    tile triplet t = t_base + p*G + g lives at (partition p, free col g)."""
    FT = R * G
    F8 = S * G
    FO = 128 * G
    TT = nc.vector.tensor_tensor
    TS = nc.vector.tensor_scalar
    mul = mybir.AluOpType.mult
    sub = mybir.AluOpType.subtract
    add = mybir.AluOpType.add

    cos_t = io.tile([P, G], FP32, name="cos_t", tag="cos")
    nc.sync.dma_start(out=cos_t, in_=bass.AP(cos_h, t_base, [[G, P], [1, G]]))
    d_t = io.tile([P, G], FP32, name="d_t", tag="d")
    nc.sync.dma_start(out=d_t, in_=bass.AP(dg_h, t_base, [[G, P], [1, G]]))

    # ---- per-triplet scalar A = env(ds) * norm / d ----
    rec_t = work.tile([P, G], FP32, name="rec_t", tag="rec")
    scr_t = work.tile([P, G], FP32, name="scr_t", tag="scr")
    nc.vector.reciprocal_approx_accurate(out=rec_t, in_=d_t, scratch=scr_t)
    ds_t = work.tile([P, G], FP32, name="ds_t", tag="ds")
    nc.vector.tensor_scalar_mul(ds_t, d_t, 1.0 / CUTOFF)
    p1_t = work.tile([P, G], FP32, name="p1_t", tag="p1")
    TT(out=p1_t, in0=ds_t, in1=ds_t, op=mul)            # ds^2
    p2_t = work.tile([P, G], FP32, name="p2_t", tag="p2")
    TT(out=p2_t, in0=p1_t, in1=p1_t, op=mul)            # ds^4
    TT(out=p2_t, in0=p2_t, in1=ds_t, op=mul)            # ds^5
    u_t = work.tile([P, G], FP32, name="u_t", tag="u")
    TS(out=u_t, in0=ds_t, scalar1=ENV_C, scalar2=ENV_B, op0=mul, op1=add)
    TT(out=u_t, in0=u_t, in1=ds_t, op=mul)              # c*ds^2 + b*ds
    nc.vector.tensor_scalar_add(u_t, u_t, ENV_A)        # + a
    TT(out=p2_t, in0=p2_t, in1=u_t, op=mul)             # ds^5*(a+b ds+c ds^2)
    TS(out=p2_t, in0=p2_t, scalar1=1.0, scalar2=NORM, op0=add, op1=mul)
    a_t = work.tile([P, G], FP32, name="a_t", tag="A")
    TT(out=a_t, in0=p2_t, in1=rec_t, op=mul)            # env * norm / d

    # ---- 16 sines: sin(pi*(r+1)*d/6), layout [p, r*G+g] ----
    h_t = work.tile([P, FT], FP32, name="h_t", tag="h")
    d_b = d_t.unsqueeze(1).broadcast_to([P, R, G])
    r_b = rc_t.unsqueeze(2).broadcast_to([P, R, G])
    TT(out=h_t.rearrange("p (r g) -> p r g", r=R), in0=d_b, in1=r_b, op=mul)
    k_t = work.tile([P, FT], FP32, name="k_t", tag="k")
    TS(out=k_t, in0=h_t, scalar1=MAGIC, scalar2=MAGIC, op0=add, op1=sub)
    TT(out=k_t, in0=h_t, in1=k_t, op=sub)               # m = h - k in [-.5,.5]
    sin_t = work.tile([P, FT], FP32, name="sin_t", tag="sinv")
    nc.scalar.activation(sin_t, k_t, mybir.ActivationFunctionType.Sin,
                         scale=TWO_PI)

    # ---- 8 gaussians * A, layout [p, s*G+g] ----
    c1_t = work.tile([P, F8], FP32, name="c1_t", tag="c1")
    cos_b = cos_t.unsqueeze(1).broadcast_to([P, S, G])
    so_b = so_t.unsqueeze(2).broadcast_to([P, S, G])
    TT(out=c1_t.rearrange("p (s g) -> p s g", s=S), in0=cos_b, in1=so_b,
       op=sub)
    nc.scalar.activation(c1_t, c1_t, mybir.ActivationFunctionType.Square,
                         scale=SQS)
    nc.scalar.activation(c1_t, c1_t, mybir.ActivationFunctionType.Exp,
                         scale=-1.0)
    a_b = a_t.unsqueeze(1).broadcast_to([P, S, G])
    TT(out=c1_t.rearrange("p (s g) -> p s g", s=S),
       in0=c1_t.rearrange("p (s g) -> p s g", s=S), in1=a_b, op=mul)

    # ---- outer product: out[p, g*128 + s*16 + r] = cbfA[p,s*G+g]*sin[p,r*G+g]
    out_t = outp.tile([P, FO], FP32, name="out_t", tag="out")
    sin_gr = bass.AP(sin_t.tensor, sin_t.offset, [sin_t.ap[0], [1, G], [G, R]])
    for s in range(S):
        o_s = bass.AP(out_t.tensor, out_t.offset + s * R,
                      [out_t.ap[0], [128, G], [1, R]])
        c_s = bass.AP(c1_t.tensor, c1_t.offset + s * G,
                      [c1_t.ap[0], [1, G], [0, R]])
        eng = nc.gpsimd if s in GPSIMD_SLICES else nc.vector
        eng.tensor_tensor(out=o_s, in0=c_s, in1=sin_gr, op=mul)

    nc.sync.dma_start(
        out=bass.AP(out_h, t_base * 128, [[G * 128, P], [1, G * 128]]),
        in_=out_t)


def build(n_main=N_MAIN, g_tail=G_TAIL):
    """Build the SPMD program. Returns (nc, t_pad)."""
    t_pad = n_main * P * G_MAIN + P * g_tail
    nc = bacc.Bacc("TRN2", target_bir_lowering=False, debug=False,
                   enable_asserts=False, num_devices=N_CORES)
    dg_h = nc.dram_tensor("dg_in", [t_pad], FP32, kind="ExternalInput")
    cos_h = nc.dram_tensor("cos_in", [t_pad], FP32, kind="ExternalInput")
    rc_h = nc.dram_tensor("rcoef", [R], FP32, kind="ExternalInput")
    so_h = nc.dram_tensor("soff", [S], FP32, kind="ExternalInput")
    out_h = nc.dram_tensor("out3", [t_pad, 128], FP32, kind="ExternalOutput")

    with tile.TileContext(nc) as tc:
        with (
            tc.tile_pool(name="const", bufs=1) as cpool,
            tc.tile_pool(name="io", bufs=3) as io,
            tc.tile_pool(name="work", bufs=2) as work,
            tc.tile_pool(name="outp", bufs=2) as outp,
        ):
            rc_t = cpool.tile([P, R], FP32, name="rc_t", tag="rc")
            nc.sync.dma_start(out=rc_t, in_=bass.AP(rc_h, 0, [[0, P], [1, R]]))
            so_t = cpool.tile([P, S], FP32, name="so_t", tag="so")
            nc.sync.dma_start(out=so_t, in_=bass.AP(so_h, 0, [[0, P], [1, S]]))

            for n in range(n_main):
                _emit_tile(nc, io, work, outp, rc_t, so_t,
                           dg_h, cos_h, out_h, n * P * G_MAIN, G_MAIN)
            if g_tail:
                _emit_tile(nc, io, work, outp, rc_t, so_t,
                           dg_h, cos_h, out_h, n_main * P * G_MAIN, g_tail)
    nc.compile()
    return nc, t_pad


def host_consts():
    rc = (np.arange(1, R + 1, dtype=np.float64) / 12.0).astype(np.float32)
    so = np.linspace(-1.0, 1.0, S).astype(np.float32)
    return rc, so


_NC = None
TRACE = False
_LAST_RESULTS = None


def kernel(D_ca, cos_phi_cab, id3_ca):
    global _NC, _LAST_RESULTS
    D = np.asarray(D_ca, dtype=np.float32).reshape(-1)
    cosp = np.asarray(cos_phi_cab, dtype=np.float32).reshape(-1)
    idx = np.asarray(id3_ca).reshape(-1).astype(np.int64)
    assert D.shape == (N_EDGES,) and cosp.shape == (N_TRIPLETS,)
    dgath = D[idx]  # edge -> triplet lookup (see module docstring)

    if _NC is None:
        _NC = build()
    nc, t_pad = _NC

    rc, so = host_consts()
    in_maps = []
    for c in range(N_CORES):
        lo = c * T_CORE
        cos_c = np.zeros(t_pad, np.float32)
        cos_c[:T_CORE] = cosp[lo:lo + T_CORE]
        dg_c = np.ones(t_pad, np.float32)
        dg_c[:T_CORE] = dgath[lo:lo + T_CORE]
        in_maps.append({"dg_in": dg_c, "cos_in": cos_c,
                        "rcoef": rc, "soff": so})

    kwargs = {}
    if TRACE:
        kwargs = dict(trace=True, trace_cores=[0])
    res = run_bass_kernel_spmd(nc, in_maps, list(range(N_CORES)), **kwargs)
    _LAST_RESULTS = res

    out = np.empty((N_TRIPLETS, 128), np.float32)
    for c in range(N_CORES):
        out[c * T_CORE:(c + 1) * T_CORE] = res.results[c]["out3"][:T_CORE]
    return out


# revision 5
# speedup vs baseline: 1.6187x; 1.6187x over previous
"""Trainium2 Bass kernel: GemNet CircularBasisLayer (radial basis x circular
basis outer product), data-parallel over triplets on 8 NeuronCores.

out[t, s*16+r] = env(ds)*norm/d * sin(pi*(r+1)*d/6) * exp(-6.125*(cos[t]-off_s)^2)
with d = D_ca[id3_ca[t]], ds = d/CUTOFF.

Division of labor:
- Host: shards triplets 8 ways and performs the edge->triplet index lookup
  d[t] = D_ca[id3_ca[t]] while marshaling inputs (a pure index gather, no
  arithmetic). On-device scattered gather was probed extensively: the SWDGE
  indirect-DMA path only consumes one index per output partition-run per
  instruction (~128 gathers / ~1us instruction => ~2 ms for 2M gathers), and
  its multi-index encodings mis-execute on hardware, so the index lookup is
  hoisted to input marshaling.
- Device (per core, 250k triplets): envelope-over-d polynomial + reciprocal
  (DVE), 16 sines via magic-constant range reduction (DVE) + Sin LUT (ACT),
  8 gaussians (DVE diff + ACT square/exp with norm folded into the exp bias),
  fused 16x8 outer product as one full-rate DVE op per tile (g-major layouts
  keep every inner stride 0/1), streams the [T, 128] fp32 output (128 MB/core)
  back with linear DMAs.
"""

import numpy as np

import concourse.bass as bass
import concourse.tile as tile
from concourse import bacc, mybir
from concourse.bass_utils import run_bass_kernel_spmd

# ---- problem constants (hardcoded; harness contract) ----
N_CORES = 8
N_EDGES = 400000
N_TRIPLETS = 2000000
R = 16          # num radial
S = 8           # num spherical
CUTOFF = 6.0
P = 128

# polynomial envelope, p=5, divided by d and with coefficients folded:
# env/d = 1/d + a2*d^4 + b2*d^5 + c2*d^6
ENV_A2 = float(-21.0 / CUTOFF**5)
ENV_B2 = float(35.0 / CUTOFF**6)
ENV_C2 = float(-15.0 / CUTOFF**7)
NORM = float(np.sqrt(2.0 / CUTOFF**3))
LN_NORM = float(np.log(NORM))
# gaussian smearing on [-1, 1], 8 centers
DELTA = 2.0 / (S - 1)
COEFF = 0.5 / (DELTA * DELTA)          # 6.125 (sign applied via exp scale)
SQS = float(np.sqrt(COEFF))
MAGIC = 12582912.0                     # 1.5 * 2**23: round-to-nearest trick
TWO_PI = float(2.0 * np.pi)

# ---- tiling ----
G_MAIN = 128                           # triplets per partition per main tile
T_CORE = N_TRIPLETS // N_CORES         # 250000
N_MAIN = T_CORE // (P * G_MAIN)        # 15 main tiles
_REM = T_CORE - N_MAIN * P * G_MAIN    # 4240
G_TAIL = -(-_REM // P)                 # 34
T_PAD = N_MAIN * P * G_MAIN + P * G_TAIL  # 250112 rows per core (tail padded)

FP32 = mybir.dt.float32


def _emit_tile(nc, io, work, outp, rc_t, so_t, ln_t, dg_h, cos_h, out_h, t_base, G):
    """Emit one tile covering triplets [t_base, t_base + 128*G); within the
    tile triplet t = t_base + p*G + g lives at (partition p, free col g).
    All per-triplet vectors are g-major: sin[p, g*16+r], cbf[p, g*8+s]."""
    FT = R * G
    F8 = S * G
    FO = 128 * G
    TT = nc.vector.tensor_tensor
    TS = nc.vector.tensor_scalar
    mul = mybir.AluOpType.mult
    sub = mybir.AluOpType.subtract
    add = mybir.AluOpType.add

    cos_t = io.tile([P, G], FP32, name="cos_t", tag="cos")
    nc.sync.dma_start(out=cos_t, in_=bass.AP(cos_h, t_base, [[G, P], [1, G]]))
    d_t = io.tile([P, G], FP32, name="d_t", tag="d")
    nc.sync.dma_start(out=d_t, in_=bass.AP(dg_h, t_base, [[G, P], [1, G]]))

    # ---- per-triplet scalar A = env(ds)/d = 1/d + d^4*(a2 + b2*d + c2*d^2)
    rec_t = work.tile([P, G], FP32, name="rec_t", tag="rec")
    scr_t = work.tile([P, G], FP32, name="scr_t", tag="scr")
    nc.vector.reciprocal_approx_accurate(out=rec_t, in_=d_t, scratch=scr_t)
    p1_t = work.tile([P, G], FP32, name="p1_t", tag="p1")
    TT(out=p1_t, in0=d_t, in1=d_t, op=mul)              # d^2
    TT(out=p1_t, in0=p1_t, in1=p1_t, op=mul)            # d^4
    a_t = work.tile([P, G], FP32, name="a_t", tag="A")
    TS(out=a_t, in0=d_t, scalar1=ENV_C2, scalar2=ENV_B2, op0=mul, op1=add)
    TT(out=a_t, in0=a_t, in1=d_t, op=mul)               # c2*d^2 + b2*d
    nc.vector.tensor_scalar_add(a_t, a_t, ENV_A2)       # + a2
    TT(out=a_t, in0=a_t, in1=p1_t, op=mul)              # d^4*(...)
    TT(out=a_t, in0=a_t, in1=rec_t, op=add)             # + 1/d

    # ---- 16 sines: sin(pi*(r+1)*d/6), g-major [p, g*16+r] ----
    h_t = work.tile([P, FT], FP32, name="h_t", tag="h")
    h_ap = bass.AP(h_t.tensor, h_t.offset, [h_t.ap[0], [R, G], [1, R]])
    d_ap = bass.AP(d_t.tensor, d_t.offset, [d_t.ap[0], [1, G], [0, R]])
    r_ap = bass.AP(rc_t.tensor, rc_t.offset, [rc_t.ap[0], [0, G], [1, R]])
    TT(out=h_ap, in0=d_ap, in1=r_ap, op=mul)            # h = d*(r+1)/12
    k_t = work.tile([P, FT], FP32, name="k_t", tag="k")
    TS(out=k_t, in0=h_t, scalar1=MAGIC, scalar2=MAGIC, op0=add, op1=sub)
    TT(out=k_t, in0=h_t, in1=k_t, op=sub)               # m = h - k in [-.5,.5]
    sin_t = work.tile([P, FT], FP32, name="sin_t", tag="sinv")
    nc.scalar.activation(sin_t, k_t, mybir.ActivationFunctionType.Sin,
                         scale=TWO_PI)

    # ---- 8 gaussians * A * norm, g-major [p, g*8+s] ----
    c1_t = work.tile([P, F8], FP32, name="c1_t", tag="c1")
    c1_ap = bass.AP(c1_t.tensor, c1_t.offset, [c1_t.ap[0], [S, G], [1, S]])
    cosb = bass.AP(cos_t.tensor, cos_t.offset, [cos_t.ap[0], [1, G], [0, S]])
    sob = bass.AP(so_t.tensor, so_t.offset, [so_t.ap[0], [0, G], [1, S]])
    TT(out=c1_ap, in0=cosb, in1=sob, op=sub)
    nc.scalar.activation(c1_t, c1_t, mybir.ActivationFunctionType.Square,
                         scale=SQS)
    nc.scalar.activation(c1_t, c1_t, mybir.ActivationFunctionType.Exp,
                         scale=-1.0, bias=ln_t)         # norm*exp(-c*diff^2)
    a_ap = bass.AP(a_t.tensor, a_t.offset, [a_t.ap[0], [1, G], [0, S]])
    TT(out=c1_ap, in0=c1_ap, in1=a_ap, op=mul)          # fold env/d in

    # ---- fused outer product, one full-rate DVE op:
    # out[p, g*128 + s*16 + r] = cbfA[p, g*8+s] * sin[p, g*16+r]
    out_t = outp.tile([P, FO], FP32, name="out_t", tag="out")
    o_ap = bass.AP(out_t.tensor, out_t.offset,
                   [out_t.ap[0], [128, G], [R, S], [1, R]])
    c_ap = bass.AP(c1_t.tensor, c1_t.offset,
                   [c1_t.ap[0], [S, G], [1, S], [0, R]])
    s_ap = bass.AP(sin_t.tensor, sin_t.offset,
                   [sin_t.ap[0], [R, G], [0, S], [1, R]])
    TT(out=o_ap, in0=c_ap, in1=s_ap, op=mul)

    nc.sync.dma_start(
        out=bass.AP(out_h, t_base * 128, [[G * 128, P], [1, G * 128]]),
        in_=out_t)


def build(n_main=N_MAIN, g_tail=G_TAIL):
    """Build the SPMD program. Returns (nc, t_pad)."""
    t_pad = n_main * P * G_MAIN + P * g_tail
    nc = bacc.Bacc("TRN2", target_bir_lowering=False, debug=False,
                   enable_asserts=False, num_devices=N_CORES)
    dg_h = nc.dram_tensor("dg_in", [t_pad], FP32, kind="ExternalInput")
    cos_h = nc.dram_tensor("cos_in", [t_pad], FP32, kind="ExternalInput")
    rc_h = nc.dram_tensor("rcoef", [R], FP32, kind="ExternalInput")
    so_h = nc.dram_tensor("soff", [S], FP32, kind="ExternalInput")
    out_h = nc.dram_tensor("out3", [t_pad, 128], FP32, kind="ExternalOutput")

    with tile.TileContext(nc) as tc:
        with (
            tc.tile_pool(name="const", bufs=1) as cpool,
            tc.tile_pool(name="io", bufs=3) as io,
            tc.tile_pool(name="work", bufs=2) as work,
            tc.tile_pool(name="outp", bufs=2) as outp,
        ):
            rc_t = cpool.tile([P, R], FP32, name="rc_t", tag="rc")
            nc.sync.dma_start(out=rc_t, in_=bass.AP(rc_h, 0, [[0, P], [1, R]]))
            so_t = cpool.tile([P, S], FP32, name="so_t", tag="so")
            nc.sync.dma_start(out=so_t, in_=bass.AP(so_h, 0, [[0, P], [1, S]]))
            ln_t = cpool.tile([P, 1], FP32, name="ln_t", tag="ln")
            nc.vector.memset(ln_t, LN_NORM)

            for n in range(n_main):
                _emit_tile(nc, io, work, outp, rc_t, so_t, ln_t,
                           dg_h, cos_h, out_h, n * P * G_MAIN, G_MAIN)
            if g_tail:
                _emit_tile(nc, io, work, outp, rc_t, so_t, ln_t,
                           dg_h, cos_h, out_h, n_main * P * G_MAIN, g_tail)
    nc.compile()
    return nc, t_pad


def host_consts():
    rc = (np.arange(1, R + 1, dtype=np.float64) / 12.0).astype(np.float32)
    so = np.linspace(-1.0, 1.0, S).astype(np.float32)
    return rc, so


_NC = None
TRACE = False
_LAST_RESULTS = None


def kernel(D_ca, cos_phi_cab, id3_ca):
    global _NC, _LAST_RESULTS
    D = np.asarray(D_ca, dtype=np.float32).reshape(-1)
    cosp = np.asarray(cos_phi_cab, dtype=np.float32).reshape(-1)
    idx = np.asarray(id3_ca).reshape(-1).astype(np.int64)
    assert D.shape == (N_EDGES,) and cosp.shape == (N_TRIPLETS,)
    dgath = D[idx]  # edge -> triplet lookup (see module docstring)

    if _NC is None:
        _NC = build()
    nc, t_pad = _NC

    rc, so = host_consts()
    in_maps = []
    for c in range(N_CORES):
        lo = c * T_CORE
        cos_c = np.zeros(t_pad, np.float32)
        cos_c[:T_CORE] = cosp[lo:lo + T_CORE]
        dg_c = np.ones(t_pad, np.float32)
        dg_c[:T_CORE] = dgath[lo:lo + T_CORE]
        in_maps.append({"dg_in": dg_c, "cos_in": cos_c,
                        "rcoef": rc, "soff": so})

    kwargs = {}
    if TRACE:
        kwargs = dict(trace=True, trace_cores=[0])
    res = run_bass_kernel_spmd(nc, in_maps, list(range(N_CORES)), **kwargs)
    _LAST_RESULTS = res

    out = np.empty((N_TRIPLETS, 128), np.float32)
    for c in range(N_CORES):
        out[c * T_CORE:(c + 1) * T_CORE] = res.results[c]["out3"][:T_CORE]
    return out
